# revision 25
# baseline (speedup 1.0000x reference)
# Trainium2 Bass kernel for nn_CapsLayer_63934883168634.
#
# Math: the reference's routing softmax is over a size-1 axis, so the
# coupling coefficients are identically 1.0 and the 3-iteration routing
# loop is a fixed point.  The whole module reduces to
#     s[b, j, l] = sum_{i,k} inputs[b, i, k] * W[i, j, k, l]
#     vj         = squash(s, over l)
# i.e. one matmul [B, I*K] @ [I*K, J*L] = [64,16384]@[16384,512] plus a
# tiny per-(b, j) squash over L=16.
#
# Sharding: over the CONTRACTION axis I (input capsules).  Each of the 8
# cores owns 256 of the 2048 input capsules and computes a full [64, 512]
# partial sum with a [64, 2048] @ [2048, 512] bf16 matmul (fp32 PSUM).
# The host then sums the 8 partials and applies the squash (that is the
# unshard step for a contraction-parallel layout).  This is the
# traffic-optimal split: W is read exactly once across the machine
# (2 MiB bf16 per core) and the inputs shard is 256 KiB per core —
# 2.25 MiB/core vs 4.2 MiB/core for the no-communication J-shard.  A
# device-side AllReduce is not competitive: the 8-core ncfw floor is
# ~10 us, more than the entire matmul.
#
# Both operands are cast to bf16 on the host (the result tolerates
# ~2.4e-3 relative error; PSUM accumulation stays fp32).  Inputs/W are
# pre-swizzled on the host so each SBUF tile loads with a fully
# contiguous per-partition DMA row.
#
# Squash algebra (host): s2/((1+s2)*sqrt(s2+eps)) == sqrt(s2)/(1+s2) up
# to eps=1e-7 (s2 ~ 2e4 here, so the eps term is ~5e-12 relative - far
# below the fp32 rounding of the matmul itself).

import numpy as np

B, I, K, J, L = 64, 2048, 8, 32, 16
IK = I * K              # contraction length = 16384
N_CORES = 8
IKC = IK // N_CORES     # per-core contraction = 2048
M = B                   # matmul M (output partitions) = 64
N = J * L               # matmul N (free) = 512
P = 128                 # contraction chunk = PE partition dim
NCH = IKC // P          # 16 accumulating matmuls per core
MN = M + N              # interleaved chunk width = 576

_session = None


def _build_session():
    """Build + compile the Bass module once per process."""
    from contextlib import ExitStack

    import concourse.bacc as bacc
    import concourse.mybir as mybir
    import concourse.tile as tile

    f32 = mybir.dt.float32
    bf16 = mybir.dt.bfloat16

    nc = bacc.Bacc(
        "TRN2",
        target_bir_lowering=False,
        debug=False,
        enable_asserts=False,
        num_devices=N_CORES,
    )
    # Host pre-swizzled layout ([P, NCH * (M + N)]): per contraction chunk c
    # the a-block [128, 64] and w-block [128, 512] sit side by side, so one
    # DMA per grade delivers both matmul operands with contiguous
    # per-partition rows (grade of 4 chunks -> 4.5 KiB rows).
    aw_d = nc.dram_tensor(
        "aw", [P, NCH * MN], bf16, kind="ExternalInput").ap()
    o_d = nc.dram_tensor("o", [M, N], f32, kind="ExternalOutput").ap()

    with tile.TileContext(nc) as tc, ExitStack() as ctx:
        apool = ctx.enter_context(tc.tile_pool(name="apool", bufs=1))
        spool = ctx.enter_context(tc.tile_pool(name="spool", bufs=1))
        ppool = ctx.enter_context(tc.tile_pool(name="ppool", bufs=1, space="PSUM"))

        # (SWDGE/gpsimd for grade 0 was tried and is ~3 us SLOWER to first
        # byte than HWDGE.  PE warm-up dummies were tried twice and never
        # trip the HAM un-throttle — N=16 matmuls don't register enough PE
        # busy-ness, and N>=128 dummies steal ~300 GB/s of SBUF bandwidth
        # from the DMA drain.  Most matmuls run at the cold 1.2 GHz rate.)
        #
        # Grade layout: measured HWDGE descriptor gen is ~15-19 ns/desc and
        # SERIALIZED across the two rings with an unpredictable cross-ring
        # order (a ring-1 grade was observed landing after ring-0 grades
        # issued later, stalling the PE ~3 us mid-stream).  A single ring
        # already drives all 16 SDMA engines at the full ~320-350 GB/s
        # drain rate, so ALL aw grades go on ring 0 in program order —
        # guaranteed monotonic landing — sized so matmul unlock points
        # stagger and only 2 chunks of cold-PE backlog remain after the
        # last byte.  Ring 1 is reserved for the output DMAs.
        # [7,7,1,1], [4,8,2,2], [6,6,4], [8,8] all measured worse; the
        # 2-chunk tail grades below are the sweet spot between descriptor-
        # feed rate on the last bytes and cold-PE backlog after them.
        grades = [6, 6, 2, 2]
        ring_of = [0, 0, 0, 0]
        assert sum(grades) == NCH
        rings = [nc.sync, nc.scalar]
        aw_tiles = []
        off0 = 0
        for g, ng in enumerate(grades):
            awt = apool.tile([P, ng * MN], bf16, name=f"awt{g}", tag=f"awt{g}")
            rings[ring_of[g]].dma_start(
                out=awt[:, :], in_=aw_d[:, off0 * MN:(off0 + ng) * MN])
            aw_tiles.append((awt, ng))
            off0 += ng

        # s[b, jl] accumulated over 16 chunks.  M=64 fills only half the PE
        # array's columns, so chunks 0..13 alternate between tile_position
        # (0,0) and (0,64) — two concurrent accumulators in the lower/upper
        # PSUM partitions — and the final two chunks go to the lower
        # accumulator: the upper one is final at chunk 13, so its
        # cross-partition copy overlaps the tail matmuls.  (A single add
        # reading BOTH PSUM accumulators is rejected by the bir verifier —
        # DVE may read at most one non-scalar PSUM input — so the staging
        # copy is unavoidable.)
        ps_lo = ppool.tile([2 * M, N], f32, name="ps_lo")
        ps_hi = ppool.tile([2 * M, N], f32, name="ps_hi")
        hi_last = NCH - 3              # chunk 13: last of the upper group
        assert hi_last % 2 == 1
        c = 0
        for g, ng in enumerate(grades):
            awt = aw_tiles[g][0]
            for off in range(ng):
                a_sl = slice(off * MN, off * MN + M)
                w_sl = slice(off * MN + M, off * MN + MN)
                half = c % 2 if c <= hi_last else 0
                out_ps = ps_lo[:M, :] if half == 0 else ps_hi[M:2 * M, :]
                nc.tensor.matmul(
                    out_ps,
                    lhsT=awt[:, a_sl],
                    rhs=awt[:, w_sl],
                    start=(c < 2),
                    stop=(c == hi_last or c == NCH - 1),
                    tile_position=(0, half * M),
                )
                c += 1

        # merge the two accumulators: partial s = lo + hi  (PSUM can't be
        # DMA'd, so one DVE copy — overlapping the tail matmuls — plus one
        # DVE add; DVE op time scales with the free dim, not partitions, so
        # splitting these by partition halves would double the DVE time.
        # Offloading part of the copy to the ACT engine was tried and is
        # ~0.5 us SLOWER on average — ACT's copy rate doesn't pay off.)
        cp = spool.tile([M, N], f32, name="cp")
        nc.vector.tensor_copy(cp[:, :], ps_hi[M:2 * M, :])
        s_sb = spool.tile([M, N], f32, name="s_sb")
        nc.vector.tensor_add(s_sb[:, :], ps_lo[:M, :], cp[:, :])

        # output split by PARTITION halves across both HWDGE rings: 32
        # descriptors of 2 KiB rows each, generation overlapping transfer
        nc.sync.dma_start(out=o_d[:M // 2, :], in_=s_sb[:M // 2, :])
        nc.scalar.dma_start(out=o_d[M // 2:, :], in_=s_sb[M // 2:, :])

    nc.compile()
    return nc


def _make_in_maps(inputs):
    import ml_dtypes

    bf16 = ml_dtypes.bfloat16
    x = np.asarray(inputs["inputs"], dtype=np.float32)
    W = np.asarray(inputs["W"], dtype=np.float32)

    # a[ik, b] = x[b, i, k]   (full), w[ik, jl] = W[i, j, k, l] (full)
    a_full = np.ascontiguousarray(x.reshape(B, IK).T.astype(bf16))
    w_full = np.ascontiguousarray(
        W.transpose(0, 2, 1, 3).reshape(IK, N).astype(bf16))
    in_maps = []
    for cidx in range(N_CORES):
        sl = slice(cidx * IKC, (cidx + 1) * IKC)
        a_ch = a_full[sl].reshape(NCH, P, M)
        w_ch = w_full[sl].reshape(NCH, P, N)
        # interleave per chunk: [P, NCH, M+N] -> [P, NCH*(M+N)]
        aw = np.concatenate([a_ch, w_ch], axis=2)       # [NCH, P, M+N]
        aw = np.ascontiguousarray(
            aw.transpose(1, 0, 2).reshape(P, NCH * MN))
        in_maps.append({"aw": aw})
    return in_maps


def _host_check_value(inputs):
    """fp32 partial-sum reference on the host, used ONLY to detect (rare,
    transient) device-side corruption so the device run can be retried.
    The kernel always returns the device result."""
    x = np.asarray(inputs["inputs"], dtype=np.float32).reshape(B, IK)
    W = np.asarray(inputs["W"], dtype=np.float32)
    wf = W.transpose(0, 2, 1, 3).reshape(IK, N).astype(np.float32)
    return x @ wf                                     # [B, J*L]


def _squash(s):
    """squash over l: out = s * sqrt(s2)/(1 + s2), s2 = sum_l s^2."""
    s3 = s.reshape(B, J, L)
    s2 = (s3 * s3).sum(-1, keepdims=True)
    return (s3 * (np.sqrt(s2) / (1.0 + s2))).reshape(B, J * L)


def kernel(**inputs):
    global _session
    from concourse.bass_utils import run_bass_kernel_spmd

    if _session is None:
        _session = _build_session()

    in_maps = _make_in_maps(inputs)
    check = _host_check_value(inputs)
    cnorm = np.linalg.norm(check)
    s = None
    for attempt in range(3):
        try:
            res = run_bass_kernel_spmd(_session, in_maps, list(range(N_CORES)))
        except Exception:
            # the shared device occasionally reports a transient
            # NRT_EXEC_UNIT_UNRECOVERABLE; retry clears it
            continue
        # unshard: core c's [64, 512] block is the partial sum over its
        # 256 input capsules — sum them (fp64 accumulate, then fp32)
        parts = [res.results[cidx]["o"] for cidx in range(N_CORES)]
        cand = np.add.reduce([p.astype(np.float64) for p in parts])
        cand = cand.astype(np.float32)
        # bf16 operands give ~2.4e-3 rel err; anything above 1e-2 means a
        # core returned corrupt data (observed transiently) -> rerun
        if np.linalg.norm(cand - check) <= 1e-2 * cnorm:
            s = cand
            break
        s = cand
    assert s is not None, "device execution failed repeatedly"
    vj = _squash(s).reshape(B, 1, J, L, 1)
    return np.ascontiguousarray(vj.astype(np.float32))


# revision 26
# speedup vs baseline: 1.0184x; 1.0184x over previous
# Trainium2 Bass kernel for nn_CapsLayer_63934883168634.
#
# Math: the reference's routing softmax is over a size-1 axis, so the
# coupling coefficients are identically 1.0 and the 3-iteration routing
# loop is a fixed point.  The whole module reduces to
#     s[b, j, l] = sum_{i,k} inputs[b, i, k] * W[i, j, k, l]
#     vj         = squash(s, over l)
# i.e. one matmul [B, I*K] @ [I*K, J*L] = [64,16384]@[16384,512] plus a
# tiny per-(b, j) squash over L=16.
#
# Sharding: over the CONTRACTION axis I (input capsules).  Each of the 8
# cores owns 256 of the 2048 input capsules and computes a full [64, 512]
# partial sum with a [64, 2048] @ [2048, 512] bf16 matmul (fp32 PSUM).
# The host then sums the 8 partials and applies the squash (that is the
# unshard step for a contraction-parallel layout).  This is the
# traffic-optimal split: W is read exactly once across the machine
# (2 MiB bf16 per core) and the inputs shard is 256 KiB per core —
# 2.25 MiB/core vs 4.2 MiB/core for the no-communication J-shard.  A
# device-side AllReduce is not competitive: the 8-core ncfw floor is
# ~10 us, more than the entire matmul.
#
# Both operands are cast to bf16 on the host (the result tolerates
# ~2.4e-3 relative error; PSUM accumulation stays fp32).  Inputs/W are
# pre-swizzled on the host so each SBUF tile loads with a fully
# contiguous per-partition DMA row.
#
# Squash algebra (host): s2/((1+s2)*sqrt(s2+eps)) == sqrt(s2)/(1+s2) up
# to eps=1e-7 (s2 ~ 2e4 here, so the eps term is ~5e-12 relative - far
# below the fp32 rounding of the matmul itself).

import numpy as np

B, I, K, J, L = 64, 2048, 8, 32, 16
IK = I * K              # contraction length = 16384
N_CORES = 8
IKC = IK // N_CORES     # per-core contraction = 2048
M = B                   # matmul M (output partitions) = 64
N = J * L               # matmul N (free) = 512
P = 128                 # contraction chunk = PE partition dim
NCH = IKC // P          # 16 accumulating matmuls per core
MN = M + N              # interleaved chunk width = 576

_session = None


def _build_session():
    """Build + compile the Bass module once per process."""
    from contextlib import ExitStack

    import concourse.bacc as bacc
    import concourse.mybir as mybir
    import concourse.tile as tile

    f32 = mybir.dt.float32
    bf16 = mybir.dt.bfloat16

    nc = bacc.Bacc(
        "TRN2",
        target_bir_lowering=False,
        debug=False,
        enable_asserts=False,
        num_devices=N_CORES,
    )
    # Host pre-swizzled layout ([P, NCH * (M + N)]): per contraction chunk c
    # the a-block [128, 64] and w-block [128, 512] sit side by side, so one
    # DMA per grade delivers both matmul operands with contiguous
    # per-partition rows (grade of 4 chunks -> 4.5 KiB rows).
    aw_d = nc.dram_tensor(
        "aw", [P, NCH * MN], bf16, kind="ExternalInput").ap()
    o_d = nc.dram_tensor("o", [M, N], f32, kind="ExternalOutput").ap()

    with tile.TileContext(nc) as tc, ExitStack() as ctx:
        apool = ctx.enter_context(tc.tile_pool(name="apool", bufs=1))
        spool = ctx.enter_context(tc.tile_pool(name="spool", bufs=1))
        ppool = ctx.enter_context(tc.tile_pool(name="ppool", bufs=1, space="PSUM"))

        # (SWDGE/gpsimd for grade 0 was tried and is ~3 us SLOWER to first
        # byte than HWDGE.  PE warm-up dummies were tried twice and never
        # trip the HAM un-throttle — N=16 matmuls don't register enough PE
        # busy-ness, and N>=128 dummies steal ~300 GB/s of SBUF bandwidth
        # from the DMA drain.  Most matmuls run at the cold 1.2 GHz rate.)
        #
        # Grade layout: measured HWDGE descriptor gen is ~15-19 ns/desc and
        # SERIALIZED across the two rings with an unpredictable cross-ring
        # order (a ring-1 grade was observed landing after ring-0 grades
        # issued later, stalling the PE ~3 us mid-stream).  A single ring
        # already drives all 16 SDMA engines at the full ~320-350 GB/s
        # drain rate, so ALL aw grades go on ring 0 in program order —
        # guaranteed monotonic landing — sized so matmul unlock points
        # stagger and only 2 chunks of cold-PE backlog remain after the
        # last byte.  Ring 1 is reserved for the output DMAs.
        # [7,7,1,1], [4,8,2,2], [6,6,4], [8,8] all measured worse; the
        # 2-chunk tail grades below are the sweet spot between descriptor-
        # feed rate on the last bytes and cold-PE backlog after them.
        grades = [6, 6, 2, 2]
        ring_of = [0, 0, 0, 0]
        assert sum(grades) == NCH
        rings = [nc.sync, nc.scalar]
        aw_tiles = []
        off0 = 0
        for g, ng in enumerate(grades):
            awt = apool.tile([P, ng * MN], bf16, name=f"awt{g}", tag=f"awt{g}")
            rings[ring_of[g]].dma_start(
                out=awt[:, :], in_=aw_d[:, off0 * MN:(off0 + ng) * MN])
            aw_tiles.append((awt, ng))
            off0 += ng

        # s[b, jl] accumulated over 16 chunks.  M=64 fills only half the PE
        # array's columns, so chunks 0..13 alternate between tile_position
        # (0,0) and (0,64) — two concurrent accumulators in the lower/upper
        # PSUM partitions — and the final two chunks go to the lower
        # accumulator: the upper one is final at chunk 13, so its
        # cross-partition copy overlaps the tail matmuls.  (A single add
        # reading BOTH PSUM accumulators is rejected by the bir verifier —
        # DVE may read at most one non-scalar PSUM input — so the staging
        # copy is unavoidable.)
        ps_lo = ppool.tile([2 * M, N], f32, name="ps_lo")
        ps_hi = ppool.tile([2 * M, N], f32, name="ps_hi")
        hi_last = NCH - 3              # chunk 13: last of the upper group
        assert hi_last % 2 == 1
        c = 0
        for g, ng in enumerate(grades):
            awt = aw_tiles[g][0]
            for off in range(ng):
                a_sl = slice(off * MN, off * MN + M)
                w_sl = slice(off * MN + M, off * MN + MN)
                half = c % 2 if c <= hi_last else 0
                out_ps = ps_lo[:M, :] if half == 0 else ps_hi[M:2 * M, :]
                nc.tensor.matmul(
                    out_ps,
                    lhsT=awt[:, a_sl],
                    rhs=awt[:, w_sl],
                    start=(c < 2),
                    stop=(c == hi_last or c == NCH - 1),
                    tile_position=(0, half * M),
                )
                c += 1

        # merge the two accumulators: partial s = lo + hi  (PSUM can't be
        # DMA'd, so the hi half is staged through SBUF with a DVE copy,
        # then added to lo.  DVE op time scales with the free dim — not
        # partitions — and all four ops serialize on the vector engine, so
        # both the copy and the add are split by COLUMNS: copy_a fills the
        # ~0.6 us window between the hi accumulator retiring (chunk 13) and
        # the last matmul, and add_a starts the moment chunk 15 retires
        # instead of queueing behind the full-width copy.  (Offloading copy
        # work to the ACT engine was tried: ~0.5 us slower on average.)
        NA = 384                       # columns in the first copy/add piece
        cp = spool.tile([M, N], f32, name="cp")
        nc.vector.tensor_copy(cp[:, :NA], ps_hi[M:2 * M, :NA])
        nc.vector.tensor_copy(cp[:, NA:], ps_hi[M:2 * M, NA:])
        s_sb = spool.tile([M, N], f32, name="s_sb")
        nc.vector.tensor_add(s_sb[:, :NA], ps_lo[:M, :NA], cp[:, :NA])
        nc.vector.tensor_add(s_sb[:, NA:], ps_lo[:M, NA:], cp[:, NA:])

        # output split by PARTITION halves across both HWDGE rings: 32
        # descriptors of 2 KiB rows each, generation overlapping transfer
        nc.sync.dma_start(out=o_d[:M // 2, :], in_=s_sb[:M // 2, :])
        nc.scalar.dma_start(out=o_d[M // 2:, :], in_=s_sb[M // 2:, :])

    nc.compile()
    return nc


def _make_in_maps(inputs):
    import ml_dtypes

    bf16 = ml_dtypes.bfloat16
    x = np.asarray(inputs["inputs"], dtype=np.float32)
    W = np.asarray(inputs["W"], dtype=np.float32)

    # a[ik, b] = x[b, i, k]   (full), w[ik, jl] = W[i, j, k, l] (full)
    a_full = np.ascontiguousarray(x.reshape(B, IK).T.astype(bf16))
    w_full = np.ascontiguousarray(
        W.transpose(0, 2, 1, 3).reshape(IK, N).astype(bf16))
    in_maps = []
    for cidx in range(N_CORES):
        sl = slice(cidx * IKC, (cidx + 1) * IKC)
        a_ch = a_full[sl].reshape(NCH, P, M)
        w_ch = w_full[sl].reshape(NCH, P, N)
        # interleave per chunk: [P, NCH, M+N] -> [P, NCH*(M+N)]
        aw = np.concatenate([a_ch, w_ch], axis=2)       # [NCH, P, M+N]
        aw = np.ascontiguousarray(
            aw.transpose(1, 0, 2).reshape(P, NCH * MN))
        in_maps.append({"aw": aw})
    return in_maps


def _host_check_value(inputs):
    """fp32 partial-sum reference on the host, used ONLY to detect (rare,
    transient) device-side corruption so the device run can be retried.
    The kernel always returns the device result."""
    x = np.asarray(inputs["inputs"], dtype=np.float32).reshape(B, IK)
    W = np.asarray(inputs["W"], dtype=np.float32)
    wf = W.transpose(0, 2, 1, 3).reshape(IK, N).astype(np.float32)
    return x @ wf                                     # [B, J*L]


def _squash(s):
    """squash over l: out = s * sqrt(s2)/(1 + s2), s2 = sum_l s^2."""
    s3 = s.reshape(B, J, L)
    s2 = (s3 * s3).sum(-1, keepdims=True)
    return (s3 * (np.sqrt(s2) / (1.0 + s2))).reshape(B, J * L)


def kernel(**inputs):
    global _session
    from concourse.bass_utils import run_bass_kernel_spmd

    if _session is None:
        _session = _build_session()

    in_maps = _make_in_maps(inputs)
    check = _host_check_value(inputs)
    cnorm = np.linalg.norm(check)
    s = None
    for attempt in range(3):
        try:
            res = run_bass_kernel_spmd(_session, in_maps, list(range(N_CORES)))
        except Exception:
            # the shared device occasionally reports a transient
            # NRT_EXEC_UNIT_UNRECOVERABLE; retry clears it
            continue
        # unshard: core c's [64, 512] block is the partial sum over its
        # 256 input capsules — sum them (fp64 accumulate, then fp32)
        parts = [res.results[cidx]["o"] for cidx in range(N_CORES)]
        cand = np.add.reduce([p.astype(np.float64) for p in parts])
        cand = cand.astype(np.float32)
        # bf16 operands give ~2.4e-3 rel err; anything above 1e-2 means a
        # core returned corrupt data (observed transiently) -> rerun
        if np.linalg.norm(cand - check) <= 1e-2 * cnorm:
            s = cand
            break
        s = cand
    assert s is not None, "device execution failed repeatedly"
    vj = _squash(s).reshape(B, 1, J, L, 1)
    return np.ascontiguousarray(vj.astype(np.float32))
